# revision 1
# baseline (speedup 1.0000x reference)
"""Trainium2 Bass kernel for nn_DeltaModel (histogram_binning).

Reference semantics (delta == 0, the shipped configuration):
  med[t,ch]   = lower median over N of logits[t,:,ch]          (rows 0-4 used)
  q[n,ch]     = sumsq - 0.1*sum^2  over the 10 rows            (9*var*... monotone in std)
  std_med[ch] = sqrt(median_N(q[:,ch]) / 9)
  mode[n,ch]  = (#{t<5: logits[t,n,ch] >= med[t,ch] + 1.96*std_med[ch]} >= 3)
  c           = broadcast(mode) over dim 0
  out[t,:,ch] = xs[t,ch] - logsumexp(xs[t,others(ch)])  (constant over N)

Device work is split into three SPMD launches over 8 NeuronCores:
  L1 "stats+med": per-core column shard -> q shard; plus 3 assigned full
      (t,ch) slices -> exact-to-3e-8 medians via bisection counting.
  L2 "qmed": per-core one channel of the assembled q array -> its median.
  L3 "mode": per-core column shard rows 0-4 + thresholds -> mode shard.
Host does only sharding/padding, tiny scalar combination of the reduction
results, and broadcast-view assembly of the two full outputs.
"""

import numpy as np

LAST_RUN_TIMES = []  # wall seconds of each device launch (incl. first-call compile)

N = 1_000_000
NCORES = 8
SHARD = N // NCORES            # 125000
PADW_PP = 992                  # per-partition padded columns (16 x 62)
SHARD_PAD = 128 * PADW_PP      # 126976
SLICE_FREE = 7816              # per-partition elements of a 1M slice
SLICE_PAD = 128 * SLICE_FREE   # 1000448
PAD_BIG = np.float32(1e30)
LEVELS_MED = 16
LEVELS_Q = 12
RANK = 500000.0
FACTOR = np.float32(1.96)
# Brackets are ~15+ sigma certain for iid N(0,1) inputs; the host re-derives
# any median whose bisection lands on a bracket boundary (never in practice).
MED_RANGE = (-0.02, 0.02)
Q_RANGE = (8.2, 8.5)


def _apply_tile_patch():
    """This walrus build rejects >2 sync waits on the SP Drain emitted at
    TileContext exit ("Too many sync wait commands"); keep one wait on the
    drain and move the rest onto dedicated SP nops before the barrier."""
    import concourse.tile as tile_mod
    from concourse import mybir
    from concourse.vector_clock import ScopedClock

    if getattr(tile_mod.TileContext, "_ant_drain_patched", False):
        return

    def _patched(self, tick_clock, wait_clock):
        nc = self.nc
        drain_inst = nc.sync.drain()
        wait_clock.add_sem_waits(
            drain_inst.ins, ScopedClock({None: tick_clock.global_clock})
        )
        si = drain_inst.ins.sync_info
        if si is not None and si.on_wait is not None and len(si.on_wait) > 1:
            waits = list(si.on_wait)
            drain_inst.ins.sync_info = mybir.SyncInfo(
                on_wait=waits[:1], on_update=list(si.on_update or [])
            )
            for w in waits[1:]:
                nop = nc.sync.nop()
                nop.ins.sync_info = mybir.SyncInfo(on_wait=[w], on_update=[])
        nc.all_engine_barrier()
        assert self.sems is not None
        popped = nc._tile_sem_poison_stack.pop()
        assert popped is self._sem_poison
        nc.clear_and_free_semaphores(list(self.sems.allocated().values()))
        nc.all_engine_barrier()

    tile_mod.TileContext._drain_and_barrier = _patched
    tile_mod.TileContext._ant_drain_patched = True


def _split_sync_waits(nc, maxw=1):
    """This walrus build caps per-instruction sync waits; move excess waits
    onto same-engine NoOps inserted right before the offending instruction."""
    from concourse import mybir

    for f in nc.m.functions:
        for b in f.blocks:
            new_list = []
            changed = False
            for ins in b.instructions:
                si = getattr(ins, "sync_info", None)
                if si is not None and si.on_wait and len(si.on_wait) > maxw:
                    waits = list(si.on_wait)
                    extra, keep = waits[:-maxw], waits[-maxw:]
                    for i in range(0, len(extra), maxw):
                        nop = mybir.InstNoOp(
                            name=f"{ins.name}-wsplit{i}", ins=[], outs=[]
                        )
                        nop.engine = ins.engine
                        nop.sync_info = mybir.SyncInfo(
                            on_wait=extra[i:i + maxw], on_update=[]
                        )
                        new_list.append(nop)
                        changed = True
                    ins.sync_info = mybir.SyncInfo(
                        on_wait=keep, on_update=list(si.on_update or [])
                    )
                new_list.append(ins)
            if changed:
                b.instructions = new_list


def _bisect_median(nc, pool, psum, ones, data_tiles, state_tiles, junk, levels,
                   rank, n_padded, act_slices=(), sgn_junk=None):
    """Shared bisection loop: for each slice s, refine [lo, lo+2h) containing
    the rank-`rank` smallest element of data_tiles[s] (n_padded elements with
    pads at +1e30).  state cols: 0=lo 1=h 2=mid 3=acc 4=nmid (all [128,1]).
    Slices in act_slices count on the Scalar engine via sign-sums (ties count
    half, shifting the located interval by at most one float step - harmless
    at our tolerance); the rest count on the Vector engine."""
    from concourse import mybir

    S = len(data_tiles)
    maskt = pool.tile([128, S], mybir.dt.int32, name="maskt")
    # count(x < mid) < rank  <=>  sum(sign(x - mid)) > n_padded - 2*rank
    sgn_thresh = float(n_padded - 2 * rank)
    for _ in range(levels):
        for s in range(S):
            st = state_tiles[s]
            lo, h, mid = st[:, 0:1], st[:, 1:2], st[:, 2:3]
            acc, cmp = st[:, 3:4], maskt[:, s:s + 1]
            nc.vector.tensor_tensor(out=mid, in0=lo, in1=h, op=mybir.AluOpType.add)
            tot = psum.tile([128, 1], mybir.dt.float32, tag=f"tot{s}", name=f"tot{s}")
            if s in act_slices:
                nmid = st[:, 4:5]
                nc.vector.scalar_tensor_tensor(
                    out=nmid, in0=lo, scalar=-1.0, in1=h,
                    op0=mybir.AluOpType.mult, op1=mybir.AluOpType.subtract,
                )
                nc.scalar.activation(
                    out=sgn_junk, in_=data_tiles[s],
                    func=mybir.ActivationFunctionType.Sign,
                    bias=nmid, scale=1.0, accum_out=acc,
                )
                nc.tensor.matmul(tot, lhsT=ones, rhs=acc, start=True, stop=True)
                nc.vector.tensor_scalar(
                    out=cmp, in0=tot, scalar1=sgn_thresh, scalar2=None,
                    op0=mybir.AluOpType.is_gt,
                )
            else:
                nc.vector.tensor_scalar(
                    out=junk, in0=data_tiles[s], scalar1=mid, scalar2=None,
                    op0=mybir.AluOpType.is_lt, op1=mybir.AluOpType.add,
                    accum_out=acc,
                )
                nc.tensor.matmul(tot, lhsT=ones, rhs=acc, start=True, stop=True)
                nc.vector.tensor_scalar(
                    out=cmp, in0=tot, scalar1=rank, scalar2=None,
                    op0=mybir.AluOpType.is_lt,
                )
            # where the median is above mid: lo <- mid
            nc.vector.copy_predicated(out=lo, mask=cmp, data=mid)
            nc.vector.tensor_scalar(
                out=h, in0=h, scalar1=0.5, scalar2=None, op0=mybir.AluOpType.mult
            )


def build_l1(nslices=3, slice_free=SLICE_FREE, padw_pp=PADW_PP, nrows=10,
             levels=LEVELS_MED, rank=RANK, use_act=True, pe_stats=True,
             split_waits=True):
    """L1: column-shard stats (q = sumsq - 0.1*sum^2) + bisection medians of
    `nslices` full slices."""
    import concourse.bass as bass
    import concourse.tile as tile
    from concourse import mybir

    _apply_tile_patch()
    chunk_pp = padw_pp // 16
    qw = padw_pp * 4
    nc = bass.Bass("TRN2", target_bir_lowering=False, debug=False, num_devices=1)
    shard = nc.dram_tensor("shardpad", [nrows, 128 * padw_pp, 4], mybir.dt.float32,
                           kind="ExternalInput").ap()
    slices = nc.dram_tensor("slices", [nslices, 128 * slice_free], mybir.dt.float32,
                            kind="ExternalInput").ap()
    ranges = nc.dram_tensor("ranges", [nslices, 2], mybir.dt.float32,
                            kind="ExternalInput").ap()
    identd = nc.dram_tensor("ident", [128, 128], mybir.dt.float32,
                            kind="ExternalInput").ap()
    qvar = nc.dram_tensor("qvar", [128, qw], mybir.dt.float32,
                          kind="ExternalOutput").ap()
    med = nc.dram_tensor("med", [1, nslices], mybir.dt.float32,
                         kind="ExternalOutput").ap()

    with tile.TileContext(nc) as tc:
        with tc.tile_pool(name="sl", bufs=1) as slpool, \
             tc.tile_pool(name="stream", bufs=2) as stream, \
             tc.tile_pool(name="scr", bufs=1) as scr, \
             tc.tile_pool(name="stat", bufs=1) as stat, \
             tc.tile_pool(name="small", bufs=1) as small, \
             tc.tile_pool(name="ps", bufs=1, space="PSUM") as psum, \
             tc.tile_pool(name="pstat", bufs=2, space="PSUM") as pstat:
            ones = small.tile([128, 128], mybir.dt.float32)
            nc.vector.memset(ones, 1.0)
            ident = small.tile([128, 128], mybir.dt.float32)
            nc.sync.dma_start(out=ident, in_=identd)
            junk = small.tile([128, slice_free], mybir.dt.bfloat16, name="junk")
            sgnj = small.tile([128, slice_free], mybir.dt.bfloat16, name="sgnj")

            state_all = small.tile([128, 8 * nslices], mybir.dt.float32)
            data_tiles, state_tiles = [], []
            for s in range(nslices):
                d = slpool.tile([128, slice_free], mybir.dt.float32, tag=f"d{s}", name=f"d{s}")
                nc.sync.dma_start(
                    out=d, in_=slices[s].rearrange("(p f) -> p f", p=128)
                )
                st = state_all[:, 8 * s:8 * s + 8]
                nc.sync.dma_start(
                    out=st[:, 0:2],
                    in_=bass.AP(tensor=ranges.tensor, offset=s * 2,
                                ap=[[0, 128], [1, 2]]),
                )
                data_tiles.append(d)
                state_tiles.append(st)

            _bisect_median(nc, small, psum, ones, data_tiles, state_tiles,
                           junk, levels, rank, n_padded=128 * slice_free,
                           act_slices=(2,) if (use_act and nslices > 2) else (),
                           sgn_junk=sgnj)

            medt = small.tile([1, nslices], mybir.dt.float32)
            for s in range(nslices):
                st = state_tiles[s]
                nc.vector.tensor_tensor(out=medt[:, s:s + 1], in0=st[0:1, 0:1],
                                        in1=st[0:1, 1:2], op=mybir.AluOpType.add)
            nc.sync.dma_start(out=med, in_=medt)

            # ---- stats over the 10 rows ----
            free = chunk_pp * 4
            for it in range(16):
                ld = stream.tile([128, nrows, free], mybir.dt.float32, tag="ld")
                src = bass.AP(
                    tensor=shard.tensor,
                    offset=it * chunk_pp * 4,
                    ap=[[padw_pp * 4, 128], [128 * padw_pp * 4, nrows],
                        [4, chunk_pp], [1, 4]],
                )
                nc.sync.dma_start(out=ld.rearrange("p t (c k) -> p t c k", k=4), in_=src)
                sq = scr.tile([128, nrows, free], mybir.dt.float32, tag="scr",
                              name="sq")
                nc.scalar.activation(out=sq, in_=ld,
                                     func=mybir.ActivationFunctionType.Square)
                if pe_stats:
                    sum_acc = pstat.tile([128, free], mybir.dt.float32, tag="sum",
                                         name="sum_ps")
                    ssq_acc = pstat.tile([128, free], mybir.dt.float32, tag="ssq",
                                         name="ssq_ps")
                    for t in range(nrows):
                        nc.tensor.matmul(sum_acc, lhsT=ident, rhs=ld[:, t, :],
                                         start=(t == 0), stop=(t == nrows - 1))
                    for t in range(nrows):
                        nc.tensor.matmul(ssq_acc, lhsT=ident, rhs=sq[:, t, :],
                                         start=(t == 0), stop=(t == nrows - 1))
                else:
                    sum_acc = stat.tile([128, free], mybir.dt.float32, tag="sum")
                    ssq_acc = stat.tile([128, free], mybir.dt.float32, tag="ssq")
                    nc.vector.tensor_copy(sum_acc, ld[:, 0, :])
                    for t in range(1, nrows):
                        nc.vector.tensor_tensor(out=sum_acc, in0=sum_acc,
                                                in1=ld[:, t, :], op=mybir.AluOpType.add)
                    nc.vector.tensor_copy(ssq_acc, sq[:, 0, :])
                    for t in range(1, nrows):
                        nc.vector.tensor_tensor(out=ssq_acc, in0=ssq_acc,
                                                in1=sq[:, t, :], op=mybir.AluOpType.add)
                t1 = stat.tile([128, free], mybir.dt.float32, tag="t1")
                # sum^2 via ACT Square: single PSUM read, exact x*x
                nc.scalar.activation(out=t1, in_=sum_acc,
                                     func=mybir.ActivationFunctionType.Square)
                nc.vector.scalar_tensor_tensor(
                    out=t1, in0=t1, scalar=-0.1, in1=ssq_acc,
                    op0=mybir.AluOpType.mult, op1=mybir.AluOpType.add,
                )
                nc.sync.dma_start(out=qvar[:, it * free:(it + 1) * free], in_=t1)
    if split_waits:
        _split_sync_waits(nc)
    return nc


def build_l2(slice_free=SLICE_FREE, levels=LEVELS_Q, rank=RANK,
             split_waits=True):
    """L2: median of one q channel per core."""
    import concourse.bass as bass
    import concourse.tile as tile
    from concourse import mybir

    _apply_tile_patch()
    nc = bass.Bass("TRN2", target_bir_lowering=False, debug=False, num_devices=1)
    qslice = nc.dram_tensor("qslice", [1, 128 * slice_free], mybir.dt.float32,
                            kind="ExternalInput").ap()
    qrange = nc.dram_tensor("qrange", [1, 2], mybir.dt.float32,
                            kind="ExternalInput").ap()
    qmed = nc.dram_tensor("qmed", [1, 1], mybir.dt.float32,
                          kind="ExternalOutput").ap()

    with tile.TileContext(nc) as tc:
        with tc.tile_pool(name="sl", bufs=1) as slpool, \
             tc.tile_pool(name="small", bufs=1) as small, \
             tc.tile_pool(name="ps", bufs=2, space="PSUM") as psum:
            ones = small.tile([128, 128], mybir.dt.float32)
            nc.vector.memset(ones, 1.0)
            junk = small.tile([128, slice_free], mybir.dt.bfloat16)
            d = slpool.tile([128, slice_free], mybir.dt.float32)
            nc.sync.dma_start(out=d, in_=qslice[0].rearrange("(p f) -> p f", p=128))
            st = small.tile([128, 8], mybir.dt.float32)
            nc.vector.memset(st, 0.0)
            nc.sync.dma_start(
                out=st[:, 0:2],
                in_=bass.AP(tensor=qrange.tensor, offset=0, ap=[[0, 128], [1, 2]]),
            )
            _bisect_median(nc, small, psum, ones, [d], [st], junk, levels, rank,
                           n_padded=128 * slice_free)
            medt = small.tile([1, 1], mybir.dt.float32)
            nc.vector.tensor_tensor(out=medt, in0=st[0:1, 0:1], in1=st[0:1, 1:2],
                                    op=mybir.AluOpType.add)
            nc.sync.dma_start(out=qmed, in_=medt)
    if split_waits:
        _split_sync_waits(nc)
    return nc


def build_l3(padw_pp=PADW_PP, nrows=5, need=3.0, split_waits=True):
    """L3: mode shard = (#rows with x >= th[t,ch]) >= need."""
    import concourse.bass as bass
    import concourse.tile as tile
    from concourse import mybir

    _apply_tile_patch()
    chunk_pp = padw_pp // 8
    qw = padw_pp * 4
    nc = bass.Bass("TRN2", target_bir_lowering=False, debug=False, num_devices=1)
    shard = nc.dram_tensor("shardpad", [10, 128 * padw_pp, 4], mybir.dt.float32,
                           kind="ExternalInput").ap()
    th = nc.dram_tensor("th", [nrows, 4], mybir.dt.float32,
                        kind="ExternalInput").ap()
    modeo = nc.dram_tensor("mode", [128, qw], mybir.dt.float32,
                           kind="ExternalOutput").ap()

    with tile.TileContext(nc) as tc:
        with tc.tile_pool(name="stream", bufs=3) as stream, \
             tc.tile_pool(name="acc", bufs=2) as accpool, \
             tc.tile_pool(name="small", bufs=1) as small:
            thb = small.tile([128, nrows * 4], mybir.dt.float32)
            nc.sync.dma_start(
                out=thb,
                in_=bass.AP(tensor=th.tensor, offset=0, ap=[[0, 128], [1, nrows * 4]]),
            )
            free = chunk_pp * 4
            for it in range(8):
                ld = stream.tile([128, nrows, free], mybir.dt.float32, tag="ld")
                src = bass.AP(
                    tensor=shard.tensor,
                    offset=it * chunk_pp * 4,
                    ap=[[padw_pp * 4, 128], [128 * padw_pp * 4, nrows],
                        [4, chunk_pp], [1, 4]],
                )
                nc.sync.dma_start(out=ld.rearrange("p t (c k) -> p t c k", k=4), in_=src)
                acc = accpool.tile([128, free], mybir.dt.float32, tag="acc")
                cmp = accpool.tile([128, free], mybir.dt.float32, tag="cmp")
                for t in range(nrows):
                    thv = bass.AP(tensor=thb.tensor, offset=thb.offset + t * 4,
                                  ap=[thb.ap[0], [0, chunk_pp], [1, 4]])
                    dst = acc if t == 0 else cmp
                    nc.vector.scalar_tensor_tensor(
                        out=dst.rearrange("p (c k) -> p c k", k=4),
                        in0=thv, scalar=0.0,
                        in1=ld[:, t, :].rearrange("p (c k) -> p c k", k=4),
                        op0=mybir.AluOpType.add, op1=mybir.AluOpType.is_le,
                    )
                    if t > 0:
                        nc.vector.tensor_tensor(out=acc, in0=acc, in1=cmp,
                                                op=mybir.AluOpType.add)
                mch = accpool.tile([128, free], mybir.dt.float32, tag="mch")
                nc.vector.tensor_scalar(out=mch, in0=acc, scalar1=need, scalar2=None,
                                        op0=mybir.AluOpType.is_ge)
                nc.sync.dma_start(out=modeo[:, it * free:(it + 1) * free], in_=mch)
    if split_waits:
        _split_sync_waits(nc)
    return nc


def _pad_shard(logits_shard, padw_pp=PADW_PP):
    """(10, SHARD, 4) -> (10, 128*padw_pp, 4) zero-padded."""
    nrows, w, chn = logits_shard.shape
    out = np.zeros((nrows, 128 * padw_pp, chn), dtype=np.float32)
    out[:, :w, :] = logits_shard
    return out


def _pad_slice(v, slice_free=SLICE_FREE):
    out = np.full(128 * slice_free, PAD_BIG, dtype=np.float32)
    out[: v.shape[0]] = v
    return out


def _trim(arr128, width, padw_pp=PADW_PP):
    """[128, padw_pp*4] core output -> (width, 4)."""
    return arr128.reshape(128 * padw_pp, 4)[:width]


def _logsumexp_f32(v):
    m = np.max(v)
    return np.float32(np.log(np.sum(np.exp(v - m, dtype=np.float32), dtype=np.float32)) + m)


def _numpy_fallback(logits, x, delta):
    logits = np.asarray(logits, dtype=np.float32)
    x = np.asarray(x, dtype=np.float32)
    delta = np.float32(delta)
    n = logits.shape[1]
    med = np.sort(logits, axis=1)[:, (n - 1) // 2, :]
    std = np.asarray(logits, dtype=np.float32).std(axis=0, ddof=1).astype(np.float32)
    std_med = np.sort(std, axis=0)[(n - 1) // 2, :]
    thresh = med[:, None, :]
    above = (logits >= thresh + FACTOR * std_med) & (logits >= thresh + delta / 2)
    cls = above.astype(np.int32)
    s = cls[:5].sum(axis=0)
    mode = (s >= 3).astype(np.float32)
    c = np.broadcast_to(mode[None], logits.shape).astype(np.float32)
    xs = np.concatenate([np.zeros((x.shape[0], 1), x.dtype), x], axis=1)
    dx = delta * c + xs[:, None, :]
    outs = []
    for i in range(4):
        oth = [j for j in range(4) if j != i]
        m = dx[..., oth].max(axis=-1)
        lse = np.log(np.sum(np.exp(dx[..., oth] - m[..., None]), axis=-1)) + m
        outs.append(dx[..., i] - lse)
    return np.stack(outs, axis=-1).astype(np.float32), c


def kernel(logits, x, delta):
    logits = np.ascontiguousarray(np.asarray(logits, dtype=np.float32))
    x = np.asarray(x, dtype=np.float32)
    dval = float(np.asarray(delta))
    if dval != 0.0 or logits.shape != (10, N, 4):
        return _numpy_fallback(logits, x, delta)

    from concourse.bass_utils import run_bass_kernel_spmd

    def _run(nc, in_maps, cores):
        # a wedged accelerator session recovers on a fresh NRT attempt
        import time as _t
        try:
            return run_bass_kernel_spmd(nc, in_maps, core_ids=cores)
        except Exception:
            _t.sleep(5)
            return run_bass_kernel_spmd(nc, in_maps, core_ids=cores)

    cores = list(range(NCORES))

    # ---------- launch 1: stats + logits medians ----------
    slice_assign = [(t, ch) for t in range(5) for ch in range(4)]
    slice_assign += [(0, 0)] * (3 * NCORES - len(slice_assign))  # dummy slots
    shard_pads = []
    in1 = []
    for c in cores:
        sh = _pad_shard(logits[:, c * SHARD:(c + 1) * SHARD, :])
        shard_pads.append(sh)
        sl = np.stack([
            _pad_slice(logits[t, :, ch]) for (t, ch) in slice_assign[3 * c:3 * c + 3]
        ])
        rg = np.array([[MED_RANGE[0], (MED_RANGE[1] - MED_RANGE[0]) / 2]] * 3,
                      dtype=np.float32)
        in1.append({"shardpad": sh, "slices": sl, "ranges": rg,
                    "ident": np.eye(128, dtype=np.float32)})
    import time as _time
    nc1 = build_l1()
    _t = _time.time()
    r1 = _run(nc1, in1, cores)
    LAST_RUN_TIMES.append(_time.time() - _t)

    qvar = np.concatenate(
        [_trim(r1.results[c]["qvar"], SHARD) for c in cores], axis=0
    )  # (N, 4)
    med = np.zeros((5, 4), dtype=np.float32)
    med_margin = 4 * (MED_RANGE[1] - MED_RANGE[0]) / 2 ** LEVELS_MED
    for idx, (t, ch) in enumerate(slice_assign[:20]):
        m = r1.results[idx // 3]["med"][0, idx % 3]
        if not (MED_RANGE[0] + med_margin < m < MED_RANGE[1] - med_margin):
            # bracket miss (never for N(0,1) inputs): exact host re-derivation
            m = np.partition(logits[t, :, ch], (N - 1) // 2)[(N - 1) // 2]
        med[t, ch] = m

    # ---------- launch 2: q medians per channel ----------
    in2 = []
    for c in cores:
        ch = c % 4
        in2.append({
            "qslice": _pad_slice(qvar[:, ch])[None, :],
            "qrange": np.array([[Q_RANGE[0], (Q_RANGE[1] - Q_RANGE[0]) / 2]],
                               dtype=np.float32),
        })
    nc2 = build_l2()
    _t = _time.time()
    r2 = _run(nc2, in2, cores)
    LAST_RUN_TIMES.append(_time.time() - _t)
    q_margin = 4 * (Q_RANGE[1] - Q_RANGE[0]) / 2 ** LEVELS_Q
    qmed = np.zeros(4, dtype=np.float32)
    for ch in range(4):
        qm = r2.results[ch]["qmed"][0, 0]
        if not (Q_RANGE[0] + q_margin < qm < Q_RANGE[1] - q_margin):
            qm = np.partition(qvar[:, ch], (N - 1) // 2)[(N - 1) // 2]
        qmed[ch] = qm
    std_med = np.sqrt(qmed / np.float32(9)).astype(np.float32)

    # ---------- launch 3: mode ----------
    th = (med + FACTOR * std_med[None, :]).astype(np.float32)
    in3 = [{"shardpad": shard_pads[c], "th": th} for c in cores]
    nc3 = build_l3()
    _t = _time.time()
    r3 = _run(nc3, in3, cores)
    LAST_RUN_TIMES.append(_time.time() - _t)
    mode = np.concatenate(
        [_trim(r3.results[c]["mode"], SHARD) for c in cores], axis=0
    )  # (N, 4) of 0.0/1.0

    # ---------- host assembly ----------
    xs = np.concatenate([np.zeros((x.shape[0], 1), np.float32), x], axis=1)
    table = np.zeros((10, 4), dtype=np.float32)
    for t in range(10):
        for i in range(4):
            oth = [j for j in range(4) if j != i]
            table[t, i] = xs[t, i] - _logsumexp_f32(xs[t, oth])
    out_full = np.broadcast_to(table[:, None, :], (10, N, 4))
    c_full = np.broadcast_to(mode[None], (10, N, 4))
    return out_full, c_full



# revision 12
# speedup vs baseline: 3.2441x; 3.2441x over previous
"""Trainium2 Bass kernel for nn_DeltaModel (histogram_binning).

Reference semantics (delta == 0, the shipped configuration):
  med[t,ch]   = lower median over N of logits[t,:,ch]   (only rows 0-4 matter)
  q[n,ch]     = sumsq - 0.1*sum^2 over the 10 rows      (= 9*unbiased var)
  std_med[ch] = sqrt(median_N(q[:,ch]) / 9)
  mode[n,ch]  = (#{t<5: logits[t,n,ch] >= med[t,ch] + 1.96*std_med[ch]} >= 3)
  c           = broadcast(mode) over dim 0
  out[t,:,ch] = xs[t,ch] - logsumexp(xs[t,others(ch)])  (constant over N)

Single fused SPMD launch over 8 NeuronCores (data-parallel over N):
  each core gets a 125000-column shard (padded to 128*992 with +1e30),
  computes q locally, then runs a 17-level joint bisection for all 24
  medians (20 logits medians + 4 q medians) where the per-level global
  rank counts come from a gpsimd AllReduce across the 8 cores.  The mode
  is computed on-device from the final thresholds and returned bit-packed
  (u8 per column, 4 channel bits).

Host does: pad+shard upload (async, overlapped with the Bass compile),
tiny bracket checks, bit-unpack, and broadcast-view assembly.
"""

import threading
import time
import numpy as np

LAST_RUN_TIMES = []   # wall seconds of the device section (compile||upload + exec)

N = 1_000_000
NCORES = 8
SHARD = N // NCORES            # 125000
PADW = 992                     # per-partition padded columns
NCOL = 128 * PADW              # 126976 per-core padded width
ROWS = 10
CW = 124                       # stats chunk (free = 496 <= 512 psum floats)
LEVELS = 17
RANK = 500000.0                # lower median of 1M = rank-500000th smallest
# Pad value: large vs the med/q brackets, but Square()-safe (no inf -> the
# identity-matmul contraction would turn 0*inf into NaN for every column).
PAD_BIG = np.float32(2.0 ** 20)
FACTOR = np.float32(1.96)
# ~15-sigma-certain brackets for iid N(0,1); host falls back if missed.
MED_RANGE = (-0.02, 0.02)
Q_RANGE = (8.2, 8.5)

_T_STRIDE = NCOL * 4           # 507904 floats between rows of the dram shard


def _apply_tile_patch():
    """This walrus build rejects >2 sync waits on the SP Drain emitted at
    TileContext exit ("Too many sync wait commands"); keep one wait on the
    drain and move the rest onto dedicated SP nops before the barrier."""
    import concourse.tile as tile_mod
    from concourse import mybir
    from concourse.vector_clock import ScopedClock

    if getattr(tile_mod.TileContext, "_ant_drain_patched", False):
        return

    def _patched(self, tick_clock, wait_clock):
        nc = self.nc
        drain_inst = nc.sync.drain()
        wait_clock.add_sem_waits(
            drain_inst.ins, ScopedClock({None: tick_clock.global_clock})
        )
        si = drain_inst.ins.sync_info
        if si is not None and si.on_wait is not None and len(si.on_wait) > 1:
            waits = list(si.on_wait)
            drain_inst.ins.sync_info = mybir.SyncInfo(
                on_wait=waits[:1], on_update=list(si.on_update or [])
            )
            for w in waits[1:]:
                nop = nc.sync.nop()
                nop.ins.sync_info = mybir.SyncInfo(on_wait=[w], on_update=[])
        nc.all_engine_barrier()
        assert self.sems is not None
        popped = nc._tile_sem_poison_stack.pop()
        assert popped is self._sem_poison
        nc.clear_and_free_semaphores(list(self.sems.allocated().values()))
        nc.all_engine_barrier()

    tile_mod.TileContext._drain_and_barrier = _patched
    tile_mod.TileContext._ant_drain_patched = True


def _split_sync_waits(nc, maxw=1):
    """This walrus build caps per-instruction sync waits; move excess waits
    onto same-engine NoOps inserted right before the offending instruction."""
    from concourse import mybir

    for f in nc.m.functions:
        for b in f.blocks:
            new_list = []
            changed = False
            for ins in b.instructions:
                si = getattr(ins, "sync_info", None)
                if si is not None and si.on_wait and len(si.on_wait) > maxw:
                    waits = list(si.on_wait)
                    extra, keep = waits[:-maxw], waits[-maxw:]
                    for i in range(0, len(extra), maxw):
                        nop = mybir.InstNoOp(
                            name=f"{ins.name}-wsplit{i}", ins=[], outs=[]
                        )
                        nop.engine = ins.engine
                        nop.sync_info = mybir.SyncInfo(
                            on_wait=extra[i:i + maxw], on_update=[]
                        )
                        new_list.append(nop)
                        changed = True
                    ins.sync_info = mybir.SyncInfo(
                        on_wait=keep, on_update=list(si.on_update or [])
                    )
                new_list.append(ins)
            if changed:
                b.instructions = new_list


def build_fused(levels=LEVELS, split_waits=True):
    import concourse.bass as bass
    import concourse.tile as tile
    from concourse import mybir

    _apply_tile_patch()
    f32 = mybir.dt.float32
    bf16 = mybir.dt.bfloat16
    Alu = mybir.AluOpType
    Act = mybir.ActivationFunctionType

    nc = bass.Bass("TRN2", target_bir_lowering=False, debug=False,
                   num_devices=NCORES)
    shard = nc.dram_tensor("shardpad", [ROWS, NCOL, 4], f32,
                           kind="ExternalInput").ap()
    lohd = nc.dram_tensor("loh", [1, 72], f32, kind="ExternalInput").ap()
    identd = nc.dram_tensor("ident", [128, 128], f32,
                            kind="ExternalInput").ap()
    modeo = nc.dram_tensor("modeu8", [128, PADW], mybir.dt.uint8,
                           kind="ExternalOutput").ap()
    medqo = nc.dram_tensor("medq", [1, 24], f32, kind="ExternalOutput").ap()
    qdumpo = nc.dram_tensor("qdump", [128, 64], f32, kind="ExternalOutput").ap()

    def dview(offset, dims):
        return bass.AP(tensor=shard.tensor, offset=offset, ap=dims)

    with tile.TileContext(nc) as tc:
        with tc.tile_pool(name="res", bufs=1) as respool, \
             tc.tile_pool(name="stream", bufs=2) as stream, \
             tc.tile_pool(name="sq", bufs=1) as sqpool, \
             tc.tile_pool(name="work", bufs=1) as work, \
             tc.tile_pool(name="small", bufs=1) as small, \
             tc.tile_pool(name="ps", bufs=2, space="PSUM") as psum, \
             tc.tile_pool(name="pst", bufs=2, space="PSUM") as psumt, \
             tc.tile_pool(name="dram", bufs=1, space="DRAM") as drp:

            ones = small.tile([128, 128], f32, name="ones")
            nc.vector.memset(ones, 1.0)
            ident = small.tile([128, 128], f32, name="ident")
            nc.sync.dma_start(out=ident, in_=identd)
            lohs = small.tile([128, 72], f32, name="lohs")
            nc.sync.dma_start(
                out=lohs,
                in_=bass.AP(tensor=lohd.tensor, offset=0,
                            ap=[[0, 128], [1, 72]]),
            )
            lo, h = lohs[:, 0:24], lohs[:, 24:48]
            rankt = lohs[:, 48:72]
            mid = small.tile([128, 24], f32, name="mid")
            cnt = small.tile([128, 32], f32, name="cnt")
            nc.vector.memset(cnt, 0.0)
            gcnt = small.tile([128, 32], f32, name="gcnt")
            ccs = small.tile([128, 32], f32, name="ccs")
            cmpt = small.tile([128, 24], mybir.dt.int32, name="cmpt")
            junk = small.tile([128, PADW], bf16, name="junk")

            # rows 0-4 resident in SBUF, interleaved (c, k) per partition
            resid = respool.tile([128, 5, PADW, 4], f32, name="resid")
            for t in range(5):
                nc.sync.dma_start(
                    out=resid[:, t],
                    in_=dview(t * _T_STRIDE, [[PADW * 4, 128], [4, PADW], [1, 4]]),
                )
            qres = respool.tile([128, PADW, 4], f32, name="qres")

            # ---- stats: q = ssq - 0.1*sum^2 over the 10 rows ----
            for i in range(8):
                st = stream.tile([128, 5, CW, 4], f32, tag="st", name="st")
                nc.sync.dma_start(
                    out=st,
                    in_=dview(5 * _T_STRIDE + i * CW * 4,
                              [[PADW * 4, 128], [_T_STRIDE, 5], [4, CW], [1, 4]]),
                )
                sq = sqpool.tile([128, 10, CW, 4], f32, tag="sq", name="sq")
                nc.scalar.activation(out=sq[:, 0:5],
                                     in_=resid[:, :, i * CW:(i + 1) * CW, :],
                                     func=Act.Square)
                nc.scalar.activation(out=sq[:, 5:10], in_=st, func=Act.Square)
                sacc = psum.tile([128, CW * 4], f32, tag="sum", name="sacc")
                qacc = psum.tile([128, CW * 4], f32, tag="ssq", name="qacc")
                for t in range(ROWS):
                    rhs = (resid[:, t, i * CW:(i + 1) * CW, :] if t < 5
                           else st[:, t - 5])
                    nc.tensor.matmul(sacc, lhsT=ident, rhs=rhs,
                                     start=(t == 0), stop=(t == ROWS - 1))
                for t in range(ROWS):
                    nc.tensor.matmul(qacc, lhsT=ident, rhs=sq[:, t],
                                     start=(t == 0), stop=(t == ROWS - 1))
                t1 = work.tile([128, CW * 4], f32, tag="t1", name="t1")
                nc.scalar.activation(out=t1, in_=sacc, func=Act.Square)
                nc.vector.scalar_tensor_tensor(
                    out=qres[:, i * CW:(i + 1) * CW, :], in0=t1, scalar=-0.1,
                    in1=qacc, op0=Alu.mult, op1=Alu.add,
                )

            nc.sync.dma_start(
                out=qdumpo,
                in_=bass.AP(tensor=qres.tensor, offset=qres.offset,
                            ap=[qres.ap[0], [1, 64]]),
            )

            # ---- joint bisection: 20 logits medians + 4 q medians ----
            bi = drp.tile([128, 32], f32, name="cc_in")
            bo = drp.tile([128, 32], f32, name="cc_out")
            for _ in range(levels):
                nc.vector.tensor_tensor(out=mid, in0=lo, in1=h, op=Alu.add)
                for t in range(5):
                    for k in range(4):
                        col = t * 4 + k
                        src = bass.AP(
                            tensor=resid.tensor,
                            offset=resid.offset + t * PADW * 4 + k,
                            ap=[resid.ap[0], [4, PADW]],
                        )
                        nc.vector.tensor_scalar(
                            out=junk, in0=src, scalar1=mid[:, col:col + 1],
                            scalar2=None, op0=Alu.is_lt, op1=Alu.add,
                            accum_out=cnt[:, col:col + 1],
                        )
                for k in range(4):
                    src = bass.AP(
                        tensor=qres.tensor, offset=qres.offset + k,
                        ap=[qres.ap[0], [4, PADW]],
                    )
                    nc.vector.tensor_scalar(
                        out=junk, in0=src, scalar1=mid[:, 20 + k:21 + k],
                        scalar2=None, op0=Alu.is_lt, op1=Alu.add,
                        accum_out=cnt[:, 20 + k:21 + k],
                    )
                tot = psumt.tile([128, 32], f32, tag="tot", name="tot")
                nc.tensor.matmul(tot, lhsT=ones, rhs=cnt, start=True, stop=True)
                nc.vector.tensor_copy(ccs, tot)
                nc.gpsimd.dma_start(out=bi, in_=ccs)
                nc.gpsimd.collective_compute(
                    "AllReduce", Alu.add,
                    replica_groups=[list(range(NCORES))],
                    ins=[bi.opt()], outs=[bo.opt()],
                )
                nc.gpsimd.dma_start(out=gcnt, in_=bo)
                nc.vector.tensor_tensor(out=cmpt, in0=gcnt[:, 0:24],
                                        in1=rankt, op=Alu.is_lt)
                nc.vector.copy_predicated(out=lo, mask=cmpt, data=mid)
                nc.vector.tensor_scalar(out=h, in0=h, scalar1=0.5,
                                        scalar2=None, op0=Alu.mult)

            # ---- thresholds + mode, on device ----
            fin = small.tile([128, 24], f32, name="fin")
            nc.vector.tensor_tensor(out=fin, in0=lo, in1=h, op=Alu.add)
            nc.sync.dma_start(out=medqo, in_=fin[0:1, :])
            sig = small.tile([128, 4], f32, name="sig")
            nc.scalar.activation(out=sig, in_=fin[:, 20:24], func=Act.Sqrt,
                                 scale=float(np.float32(1.0) / np.float32(9.0)))
            th = small.tile([128, 20], f32, name="th")
            th_v = bass.AP(tensor=th.tensor, offset=th.offset,
                           ap=[th.ap[0], [4, 5], [1, 4]])
            sig_v = bass.AP(tensor=sig.tensor, offset=sig.offset,
                            ap=[sig.ap[0], [0, 5], [1, 4]])
            med_v = bass.AP(tensor=fin.tensor, offset=fin.offset,
                            ap=[fin.ap[0], [4, 5], [1, 4]])
            nc.vector.scalar_tensor_tensor(out=th_v, in0=sig_v,
                                           scalar=float(FACTOR), in1=med_v,
                                           op0=Alu.mult, op1=Alu.add)

            acc = work.tile([128, PADW, 4], bf16, name="macc")
            cm = work.tile([128, PADW, 4], bf16, name="mcmp")
            for t in range(5):
                thv = bass.AP(tensor=th.tensor, offset=th.offset + t * 4,
                              ap=[th.ap[0], [0, PADW], [1, 4]])
                dst = acc if t == 0 else cm
                nc.vector.scalar_tensor_tensor(out=dst, in0=thv, scalar=0.0,
                                               in1=resid[:, t], op0=Alu.add,
                                               op1=Alu.is_le)
                if t:
                    nc.vector.tensor_tensor(out=acc, in0=acc, in1=cm,
                                            op=Alu.add)
            nc.vector.tensor_scalar(out=acc, in0=acc, scalar1=3.0,
                                    scalar2=None, op0=Alu.is_ge)

            def accview(k):
                return bass.AP(tensor=acc.tensor, offset=acc.offset + k,
                               ap=[acc.ap[0], [4, PADW]])

            pk = work.tile([128, PADW], bf16, name="pk")
            pk2 = work.tile([128, PADW], bf16, name="pk2")
            nc.vector.scalar_tensor_tensor(out=pk, in0=accview(1), scalar=2.0,
                                           in1=accview(0), op0=Alu.mult,
                                           op1=Alu.add)
            nc.vector.scalar_tensor_tensor(out=pk2, in0=accview(3), scalar=2.0,
                                           in1=accview(2), op0=Alu.mult,
                                           op1=Alu.add)
            nc.vector.scalar_tensor_tensor(out=pk, in0=pk2, scalar=4.0,
                                           in1=pk, op0=Alu.mult, op1=Alu.add)
            pk8 = work.tile([128, PADW], mybir.dt.uint8, name="pk8")
            nc.vector.tensor_copy(pk8, pk)
            nc.sync.dma_start(out=modeo, in_=pk8)

    if split_waits:
        _split_sync_waits(nc)
    return nc


def _make_compiled(nc):
    """AOT-compile the fused kernel as a jit(shard_map(...)) over 8 cores.
    Mirrors concourse.bass2jax.run_bass_via_pjrt but takes device-resident
    jax arrays (no host concat / re-upload) and compiles from avals so the
    walrus compile can overlap the input upload."""
    import jax
    from jax.experimental.shard_map import shard_map
    from jax.sharding import Mesh, NamedSharding, PartitionSpec
    from concourse import mybir
    from concourse.bass2jax import (_bass_exec_p, install_neuronx_cc_hook,
                                    partition_id_tensor)

    install_neuronx_cc_hook()
    assert nc.dbg_addr is None or not nc.dbg_callbacks
    partition_name = (nc.partition_id_tensor.name
                      if nc.partition_id_tensor else None)

    in_names, in_avals = [], []
    out_names, out_avals = [], []
    for alloc in nc.m.functions[0].allocations:
        if not isinstance(alloc, mybir.MemoryLocationSet):
            continue
        name = alloc.memorylocations[0].name
        shape = tuple(alloc.tensor_shape) if alloc.tensor_shape else None
        if alloc.kind == "ExternalInput":
            if name != partition_name:
                in_names.append(name)
                in_avals.append((shape, mybir.dt.np(alloc.dtype)))
        elif alloc.kind == "ExternalOutput":
            dtype = mybir.dt.np(alloc.dtype)
            out_names.append(name)
            out_avals.append(jax.core.ShapedArray(shape, dtype))

    n_params = len(in_names)
    n_outs = len(out_names)
    all_in_names = list(in_names) + list(out_names)
    if partition_name is not None:
        all_in_names.append(partition_name)

    def _body(*args):
        operands = list(args)
        if partition_name is not None:
            operands.append(partition_id_tensor())
        outs = _bass_exec_p.bind(
            *operands,
            out_avals=tuple(out_avals),
            in_names=tuple(all_in_names),
            out_names=tuple(out_names),
            lowering_input_output_aliases=(),
            sim_require_finite=True,
            sim_require_nnan=True,
            nc=nc,
        )
        return tuple(outs)

    devices = jax.devices()[:NCORES]
    mesh = Mesh(np.asarray(devices), ("core",))
    spec = NamedSharding(mesh, PartitionSpec("core"))
    in_specs = (PartitionSpec("core"),) * (n_params + n_outs)
    out_specs = (PartitionSpec("core"),) * n_outs
    donate = tuple(range(n_params, n_params + n_outs))
    sharded = jax.jit(
        shard_map(_body, mesh=mesh, in_specs=in_specs, out_specs=out_specs,
                  check_rep=False),
        donate_argnums=donate, keep_unused=True,
    )
    avals = [
        jax.ShapeDtypeStruct((NCORES * s[0],) + tuple(s[1:]), dt, sharding=spec)
        for (s, dt) in in_avals
    ] + [
        jax.ShapeDtypeStruct((NCORES * a.shape[0],) + tuple(a.shape[1:]),
                             a.dtype, sharding=spec)
        for a in out_avals
    ]
    compiled = sharded.lower(*avals).compile()
    return compiled, in_names, out_names, out_avals, spec


def _build_padded(logits):
    """(10, N, 4) -> core-major (8*10, NCOL, 4), padded with +1e30."""
    from concurrent.futures import ThreadPoolExecutor

    G = np.empty((NCORES, ROWS, NCOL, 4), np.float32)

    def fill(c):
        G[c, :, :SHARD, :] = logits[:, c * SHARD:(c + 1) * SHARD, :]
        G[c, :, SHARD:, :] = PAD_BIG

    with ThreadPoolExecutor(NCORES) as ex:
        list(ex.map(fill, range(NCORES)))
    return G.reshape(NCORES * ROWS, NCOL, 4)


def _logsumexp_f32(v):
    m = np.max(v)
    return np.float32(
        np.log(np.sum(np.exp(v - m, dtype=np.float32), dtype=np.float32)) + m
    )


def _numpy_fallback(logits, x, delta):
    logits = np.asarray(logits, dtype=np.float32)
    x = np.asarray(x, dtype=np.float32)
    delta = np.float32(delta)
    n = logits.shape[1]
    med = np.sort(logits, axis=1)[:, (n - 1) // 2, :]
    std = logits.std(axis=0, ddof=1).astype(np.float32)
    std_med = np.sort(std, axis=0)[(n - 1) // 2, :]
    thresh = med[:, None, :]
    above = (logits >= thresh + FACTOR * std_med) & (logits >= thresh + delta / 2)
    cls = above.astype(np.int32)
    s = cls[:5].sum(axis=0)
    mode = (s >= 3).astype(np.float32)
    c = np.broadcast_to(mode[None], logits.shape).astype(np.float32)
    xs = np.concatenate([np.zeros((x.shape[0], 1), x.dtype), x], axis=1)
    dx = delta * c + xs[:, None, :]
    outs = []
    for i in range(4):
        oth = [j for j in range(4) if j != i]
        m = dx[..., oth].max(axis=-1)
        lse = np.log(np.sum(np.exp(dx[..., oth] - m[..., None]), axis=-1)) + m
        outs.append(dx[..., i] - lse)
    return np.stack(outs, axis=-1).astype(np.float32), c


def _host_table(x):
    xs = np.concatenate([np.zeros((x.shape[0], 1), np.float32), x], axis=1)
    table = np.zeros((ROWS, 4), dtype=np.float32)
    for t in range(ROWS):
        for i in range(4):
            oth = [j for j in range(4) if j != i]
            table[t, i] = xs[t, i] - _logsumexp_f32(xs[t, oth])
    return table


def _device_mode(logits):
    """Run the fused device kernel; returns (mode(N,4) f32, med(5,4), qmed(4))."""
    import jax
    from jax.sharding import Mesh, NamedSharding, PartitionSpec

    state = {}

    def upload():
        devices = jax.devices()[:NCORES]
        mesh = Mesh(np.asarray(devices), ("core",))
        spec = NamedSharding(mesh, PartitionSpec("core"))
        G = _build_padded(logits)
        lo24 = [MED_RANGE[0]] * 20 + [Q_RANGE[0]] * 4
        h24 = ([(MED_RANGE[1] - MED_RANGE[0]) / 2] * 20
               + [(Q_RANGE[1] - Q_RANGE[0]) / 2] * 4)
        # Pad columns (all 10 rows == PAD_BIG) produce a deterministic q that
        # we replicate here in exact f32 to know whether pads count below the
        # q bracket; shift the q rank by the global pad count accordingly.
        v = PAD_BIG
        sumv = np.float32(10) * v
        t1v = sumv * sumv
        qaccv = np.float32(10) * (v * v)
        qpad = np.float32(np.float32(-0.1) * t1v) + qaccv
        if qpad < np.float32(Q_RANGE[0]):
            qoff = float((NCOL - SHARD) * NCORES)
        elif qpad >= np.float32(Q_RANGE[1]):
            qoff = 0.0
        else:
            raise RuntimeError(f"pad q value {qpad} inside q bracket")
        rank24 = [RANK] * 20 + [RANK + qoff] * 4
        loh = np.asarray([lo24 + h24 + rank24], np.float32)
        arrs = {
            "shardpad": G,
            "loh": np.tile(loh, (NCORES, 1)),
            "ident": np.tile(np.eye(128, dtype=np.float32), (NCORES, 1)),
            "modeu8": np.zeros((NCORES * 128, PADW), np.uint8),
            "medq": np.zeros((NCORES * 1, 24), np.float32),
            "qdump": np.zeros((NCORES * 128, 64), np.float32),
        }
        devarrs = {k: jax.device_put(v, spec) for k, v in arrs.items()}
        for v in devarrs.values():
            v.block_until_ready()
        state["dev"] = devarrs

    up = threading.Thread(target=upload)
    up.start()
    nc = build_fused()
    compiled, in_names, out_names, _, _ = _make_compiled(nc)
    up.join()
    if "dev" not in state:
        raise RuntimeError("upload failed")
    dev = state["dev"]
    args = [dev[n] for n in in_names] + [dev[n] for n in out_names]
    out_arrs = compiled(*args)
    res = {n: np.asarray(out_arrs[i]) for i, n in enumerate(out_names)}

    medq = res["medq"].reshape(NCORES, 24)[0]
    med = medq[:20].reshape(5, 4)
    qmed = medq[20:24]
    margin = 4 * (MED_RANGE[1] - MED_RANGE[0]) / 2 ** LEVELS
    qmargin = 4 * (Q_RANGE[1] - Q_RANGE[0]) / 2 ** LEVELS
    if not (np.all(med > MED_RANGE[0] + margin)
            and np.all(med < MED_RANGE[1] - margin)
            and np.all(qmed > Q_RANGE[0] + qmargin)
            and np.all(qmed < Q_RANGE[1] - qmargin)):
        raise RuntimeError(
            f"bisection bracket missed: med={med.tolist()} qmed={qmed.tolist()}"
        )

    pk = res["modeu8"].reshape(NCORES, 128 * PADW)[:, :SHARD].reshape(-1)
    bits = (pk[:, None] >> np.arange(4, dtype=np.uint8)) & np.uint8(1)
    mode = bits.astype(np.float32)          # (N, 4)
    if not (1e-6 < mode.mean() < 0.05):
        raise RuntimeError(f"implausible mode density {mode.mean():.2e}")
    return mode, med, qmed


def kernel(logits, x, delta):
    logits = np.ascontiguousarray(np.asarray(logits, dtype=np.float32))
    x = np.asarray(x, dtype=np.float32)
    dval = float(np.asarray(delta))
    if dval != 0.0 or logits.shape != (ROWS, N, 4):
        return _numpy_fallback(logits, x, delta)
    t0 = time.time()
    try:
        mode, _, _ = _device_mode(logits)
    except Exception:
        import traceback
        traceback.print_exc()
        return _numpy_fallback(logits, x, delta)
    LAST_RUN_TIMES.append(time.time() - t0)

    table = _host_table(x)
    out_full = np.broadcast_to(table[:, None, :], (ROWS, N, 4))
    c_full = np.broadcast_to(mode[None], (ROWS, N, 4))
    return out_full, c_full
